# revision 33
# baseline (speedup 1.0000x reference)
"""Trainium2 Bass kernel for nn_EstimatorQNN.

Math reduction: the reference applies a batch-independent 2x2 unitary U
(built from the 4 weights) to |psi> = [cos(th/2), sin(th/2)] with
th = x0 + x1, then returns |amp0|^2 - |amp1|^2.  By unitarity this
collapses to

    out = A*cos(th) + D*sin(th) = R*sin(th + phi)

with A = 2|U00|^2 - 1, D = 2*Re(U00*conj(U01)), R = hypot(A, D),
phi = atan2(A, D).  R/phi are scalars computed on host from the weights;
the device does the memory-bound elementwise part.

Design (vs the ~50us f32 baseline): fp16 end-to-end halves HBM traffic
(6.3 MB/core instead of 12.6 MB), and the 5-op DVE+ACT chain is fused to
3 engine-balanced ops via one new custom DVE op:

    TURNS_FRAC (DVE, 1 op):  z = (x_even + x_odd)*C1 + C0   with
        C1 = 1/2pi, C0 = phi/2pi (work in *turns*, not radians);
        k = (z + MAGIC) - MAGIC   (fp32 magic-number round-to-nearest,
        verified on HW);  f = z - k  in [-0.5, 0.5]
    Sin (ACT, 1 op):   s = Sin(2pi * f)              (in [-pi, pi])
    mul (DVE, 1 op):   y = s * R                     (tensor_scalar, 4x fp16)

All DRAM<->SBUF traffic is declared uint32 (2 packed fp16) — the DMA
engines move 2-byte elements measurably slower than 4-byte ones — and the
SBUF tiles are bitcast to fp16 for compute.  Loads ride the sync HWDGE
ring strictly in consumption order (FIFO queues: in-order completion
beats two-ring aggregate bandwidth end-to-end, A/B-measured); stores ride
the gpsimd SWDGE ring.  DVE ~13 us and ACT ~10 us busy hide under the
~13 us load stream plus ~13 us of fixed per-NEFF overhead (init barriers
+ walrus postamble that resets all 256 semaphores).  A global op plan is
linearized and every RAW hazard gets an explicit semaphore wait.  Pure
data parallel over 8 NeuronCores; host casts f32->fp16 going in,
fp16->f32 coming out.
"""

import math
from contextlib import ExitStack

import numpy as np

B_FULL = 8388608
N_CORES = 8
B_SHARD = B_FULL // N_CORES  # 1048576

TOT_COLS = B_SHARD * 2 // 128  # 16384 fp16 inputs per partition
H_TOT = TOT_COLS // 2          # 8192 outputs per partition

# per-block input columns (fp16 elems); smaller blocks at the edges for
# pipeline ramp-up/drain, bigger in the middle for low per-op overhead
# The DMA engines move 2-byte elements at only ~21 B/ns vs ~25.6 B/ns for
# 4-byte elements, so all DRAM<->SBUF traffic is declared uint32 (one u32 =
# two packed fp16) and the SBUF tiles are bitcast to fp16 for compute.
# All loads ride the sync HWDGE ring strictly in consumption order: DGE
# queues are FIFO, so one ring gives each block the earliest-possible
# completion for its consumer even though two rings have higher aggregate
# bandwidth (~420 vs ~316 B/ns) — A/B-measured faster end-to-end.  Stores
# ride the gpsimd SWDGE ring, keeping both compute engines' sequencers
# free of ~0.7us DMA_DIRECT2D issue slots.
LOAD_COLS = [1024, 2048, 2048, 2048, 2048, 2048, 2048, 1536, 1024, 512]
LOAD_RING = ["s"] * 10
MUL_GROUPS = [(0, 2), (2, 4), (4, 6), (6, 8), (8, 9), (9, 10)]
assert sum(LOAD_COLS) == TOT_COLS
N_BLOCKS = len(LOAD_COLS)
STORE_RING = ["g", "g", "g", "g", "g", "g"]

MAGIC = 12582912.0  # 1.5 * 2**23: fp32 magic-number integer round
TWO_PI = 6.283185307179586

LAST_RESULT = None


def _host_constants(weights: np.ndarray):
    w = np.asarray(weights, dtype=np.float64)

    def rx(t):
        c, s = np.cos(t / 2), np.sin(t / 2)
        return np.array([[c, -1j * s], [-1j * s, c]], dtype=np.complex128)

    def rz(t):
        return np.array(
            [[np.exp(-1j * t / 2), 0], [0, np.exp(1j * t / 2)]], dtype=np.complex128
        )

    U = np.eye(2, dtype=np.complex128)
    for i in range(len(w) // 2):
        U = rz(w[2 * i + 1]) @ rx(w[2 * i]) @ U
    A = 2.0 * abs(U[0, 0]) ** 2 - 1.0
    D = 2.0 * (U[0, 0] * np.conj(U[0, 1])).real
    R = math.hypot(A, D)
    phi = math.atan2(A, D)
    return float(R), float(phi)


# "safe" = rounding-mode-agnostic 8-stage body (f in [0,1), needs -pi Sin
# bias); "rne" = 6-stage body relying on round-to-nearest f32 adds
# (f in [-0.5,0.5], Sin bias 0).
FRAC_VARIANT = "rne"


def _register_turns_frac():
    """Define + register the TURNS_FRAC custom DVE op (documented runtime
    extension point: dve_ops.OPS + the name->row / name->spec side tables)."""
    from concourse import dve_ops
    from concourse.dve_spec import Spec, Src0, Src1, C0, C1, C2, Zero, lower
    from concourse.dve_uop import DveOpSpec

    NAME = f"TURNS_FRAC_{FRAC_VARIANT.upper()}_ANT"
    for op in dve_ops.OPS:
        if op.name == NAME:
            return op

    z = (Src0 + Src1) * C1 + C0
    k = (z + C2) - C2
    f = z - k
    if FRAC_VARIANT == "safe":
        body = f + (f < Zero)
    else:
        body = f

    def _ref(in0, in1, s0, s1, imm2):
        zz = (in0.astype(np.float32) + in1.astype(np.float32)) * s1 + s0
        kk = (zz + imm2) - imm2
        ff = zz - kk
        return ff + (ff < 0) if FRAC_VARIANT == "safe" else ff

    spec = Spec(body=body, reference=_ref)
    row = dve_ops._CUSTOM_DVE_ROW_BASE + len(dve_ops.OPS)
    shas = {}
    for ver in ("v3", "v4"):
        uops = lower(spec, ver=ver)
        shas[ver] = DveOpSpec(name=NAME, opcode=row, uops=uops, rd1_en=True).sha(ver)
    op = dve_ops.DveOp(NAME, spec, subdim=False, uops_sha=shas)
    dve_ops.OPS.append(op)
    dve_ops._SUB_OPCODE_FOR_NAME[NAME] = row
    dve_ops.CUSTOM_DVE_SPECS[NAME] = spec
    return op


def _plan_waits(plan):
    """Assign per-op semaphore waits for every RAW/WAR/WAW hazard."""
    semval = {}
    writer = {}
    readers = {}
    seen = {}
    for op in plan:
        want = {}
        for b in op["reads"]:
            if b in writer:
                s, v = writer[b]
                want[s] = max(want.get(s, 0), v)
        for b in op["writes"]:
            for s, v in readers.get(b, []):
                want[s] = max(want.get(s, 0), v)
            if b in writer:
                s, v = writer[b]
                want[s] = max(want.get(s, 0), v)
        eng_seen = seen.setdefault(op["eng"], {})
        waits = []
        for s, v in want.items():
            if eng_seen.get(s, -1) < v:
                waits.append((s, v))
                eng_seen[s] = v
        op["waits"] = waits
        semval[op["sem"]] = semval.get(op["sem"], 0) + op["inc"]
        point = (op["sem"], semval[op["sem"]])
        for b in op["writes"]:
            writer[b] = point
            readers[b] = []
        for b in op["reads"]:
            readers.setdefault(b, []).append(point)
    return plan


def _build_nc(R: float, phi: float):
    import concourse.bacc as bacc
    from concourse import mybir

    turns_frac = _register_turns_frac()

    f16 = mybir.dt.float16
    u32 = mybir.dt.uint32
    Sin = mybir.ActivationFunctionType.Sin

    nc = bacc.Bacc(
        "TRN2",
        target_bir_lowering=False,
        debug=False,
        enable_asserts=False,
        num_devices=N_CORES,
    )
    # DMA-facing tensors are uint32 (two fp16 per element) for full AXI rate
    x = nc.dram_tensor("x", [B_SHARD], u32, kind="ExternalInput").ap()
    y = nc.dram_tensor("y", [B_SHARD // 2], u32, kind="ExternalOutput").ap()
    xf = x.rearrange("(p c) -> p c", p=128)      # [128, TOT_COLS//2] u32
    yf = y.rearrange("(p c) -> p c", p=128)      # [128, H_TOT//2] u32

    lcol = [sum(LOAD_COLS[:i]) for i in range(N_BLOCKS)]  # arena col offsets
    hoff = [c // 2 for c in lcol]                         # output col offsets
    hcols = [c // 2 for c in LOAD_COLS]

    arena = nc.alloc_sbuf_tensor("arena", [128, TOT_COLS // 2], u32)
    fbuf = nc.alloc_sbuf_tensor("fbuf", [128, H_TOT], f16)
    sbuf = nc.alloc_sbuf_tensor("sbuf", [128, H_TOT], f16)
    obuf = nc.alloc_sbuf_tensor("obuf", [128, H_TOT // 2], u32)
    arena16 = arena.ap().bitcast(f16)            # [128, TOT_COLS] fp16 view
    obuf16 = obuf.ap().bitcast(f16)              # [128, H_TOT] fp16 view

    if FRAC_VARIANT == "safe":
        # f in [0,1): Sin needs bias -pi, a [128,1] const AP; register it the
        # same way the Bass constructor registers 0.0/1.0 (memset + barrier)
        bias_t = nc.alloc_sbuf_tensor("bias_mpi", [128, 1], mybir.dt.float32)
        nc.gpsimd.memset(bias_t.ap(), -math.pi)
        nc.all_engine_barrier()
        sin_bias = bias_t.ap()
        C0 = phi / TWO_PI + 0.5
    else:
        # f in [-0.5,0.5]: Sin(2*pi*f) directly, zero bias (pre-registered)
        sin_bias = 0.0
        C0 = phi / TWO_PI
    C1 = 1.0 / TWO_PI

    # ---- phase 1: global plan --------------------------------------------
    def op(eng, kind, i, reads, writes, sem, inc=1):
        return dict(eng=eng, kind=kind, i=i, reads=reads, writes=writes,
                    sem=sem, inc=inc)

    # (block, u32 col start, u32 col end, ring, token) load parts
    load_parts = []
    for j in range(N_BLOCKS):
        cu0, cu1 = lcol[j] // 2, (lcol[j] + LOAD_COLS[j]) // 2
        load_parts.append((j, cu0, cu1, LOAD_RING[j], f"t{j}a"))

    def blk_tokens(b):
        return [f"t{b}a"]

    plan = []
    for pi, (j, cu0, cu1, ring, tok) in enumerate(load_parts):
        plan.append(op(ring, "load", pi, [], [tok], f"l{pi}", 16))
    # mul group g is planned after sin[min(hi, N-1)]: one block of slack so
    # the vector stream rarely stalls waiting for ACT (plan order is both
    # the topological order for _plan_waits and per-engine program order)
    groups_at = {}
    for g, (lo, hi) in enumerate(MUL_GROUPS):
        groups_at.setdefault(min(hi, N_BLOCKS - 1), []).append(g)
    for b in range(N_BLOCKS):
        plan.append(op("v", "frac", b, blk_tokens(b), [f"f{b}"], "vq"))
        plan.append(op("a", "sin", b, [f"f{b}"], [f"s{b}"], "aq"))
        for g in groups_at.get(b, []):
            lo, hi = MUL_GROUPS[g]
            plan.append(op("v", "mul", g,
                           [f"s{bb}" for bb in range(lo, hi)], [f"o{g}"], "vq"))
            plan.append(op(STORE_RING[g], "store", g, [f"o{g}"], [], f"os{g}", 16))

    _plan_waits(plan)

    # ---- phase 2: emit per-engine streams --------------------------------
    with ExitStack() as ctx:
        sems = {}
        for o in plan:
            if o["sem"] not in sems:
                sems[o["sem"]] = ctx.enter_context(nc.semaphore(o["sem"]))
        block = ctx.enter_context(nc.Block())

        def emit(o, eng):
            for s, v in o["waits"]:
                eng.wait_ge(sems[s], v)
            i = o["i"]
            k = o["kind"]
            if k == "load":
                _, cu0, cu1, _, _ = load_parts[i]
                inst = eng.dma_start(
                    arena.ap()[:, cu0:cu1], xf[:, cu0:cu1]
                )
            elif k == "store":
                lo, hi = MUL_GROUPS[i]
                h0 = hoff[lo]
                h1 = hoff[hi - 1] + hcols[hi - 1]
                inst = eng.dma_start(
                    yf[:, h0 // 2 : h1 // 2], obuf.ap()[:, h0 // 2 : h1 // 2]
                )
            elif k == "frac":
                t = arena16[:, lcol[i] : lcol[i] + LOAD_COLS[i]]
                h = hcols[i]
                inst = nc.vector._custom_dve(
                    turns_frac,
                    out=fbuf.ap()[:, hoff[i] : hoff[i] + h],
                    in0=t[:, 0 : 2 * h : 2],
                    in1=t[:, 1 : 2 * h : 2],
                    s0=C0,
                    s1=C1,
                    imm2=MAGIC,
                )
            elif k == "sin":
                h = hcols[i]
                inst = nc.scalar.activation(
                    sbuf.ap()[:, hoff[i] : hoff[i] + h],
                    fbuf.ap()[:, hoff[i] : hoff[i] + h],
                    Sin,
                    bias=sin_bias,
                    scale=TWO_PI,
                )
            elif k == "mul":
                lo, hi = MUL_GROUPS[i]
                h0 = hoff[lo]
                h1 = hoff[hi - 1] + hcols[hi - 1]
                inst = nc.vector.tensor_scalar_mul(
                    obuf16[:, h0:h1], sbuf.ap()[:, h0:h1], R
                )
            else:
                raise AssertionError(k)
            inst.then_inc(sems[o["sem"]], o["inc"])

        @block.sync
        def _(sync):
            for o in plan:
                if o["eng"] == "s":
                    emit(o, sync)
            for g in range(len(MUL_GROUPS)):
                if STORE_RING[g] == "s":
                    sync.wait_ge(sems[f"os{g}"], 16)

        @block.vector
        def _(vector):
            for o in plan:
                if o["eng"] == "v":
                    emit(o, vector)

        @block.scalar
        def _(scalar):
            for o in plan:
                if o["eng"] == "a":
                    emit(o, scalar)

        @block.gpsimd
        def _(gpsimd):
            for o in plan:
                if o["eng"] == "g":
                    emit(o, gpsimd)
            for g in range(len(MUL_GROUPS)):
                if STORE_RING[g] == "g":
                    gpsimd.wait_ge(sems[f"os{g}"], 16)

    nc.compile()
    return nc


def kernel(inputs: np.ndarray, weights: np.ndarray, _trace: bool = False) -> np.ndarray:
    global LAST_RESULT
    from concourse.bass_utils import run_bass_kernel_spmd

    inputs = np.asarray(inputs)
    assert inputs.shape == (B_FULL, 2), inputs.shape

    R, phi = _host_constants(weights)
    nc = _build_nc(R, phi)

    x32 = np.ascontiguousarray(inputs.astype(np.float16)).view(np.uint32)
    x32 = x32.reshape(B_FULL)
    in_maps = [
        {"x": x32[c * B_SHARD : (c + 1) * B_SHARD]} for c in range(N_CORES)
    ]
    res = run_bass_kernel_spmd(
        nc, in_maps, core_ids=list(range(N_CORES)), trace=_trace
    )
    LAST_RESULT = res
    out32 = np.concatenate([r["y"] for r in res.results], axis=0)
    out = out32.view(np.float16).astype(np.float32).reshape(B_FULL, 1)
    return out
